# revision 1
# baseline (speedup 1.0000x reference)
"""Causal self-attention (B=4, T=2048, C=1024, H=16, D=64) on 8 trn2 cores.

Sharding: data-parallel over B (4) x tensor-parallel over head-halves (2).
Core c handles batch c//2 with heads [8*(c%2), 8*(c%2)+8). Each core emits a
partial projection output [2048, 1024]; host sums the two head-half partials
per batch and adds the (bv @ Wp + bp) correction row.

Device layout highlights:
 - all matmuls in float32r (full PE rate, ~1e-4 rel err)
 - x is pre-transposed on host, so Q^T/K^T/V all come out of natural-layout
   matmuls; S^T = K^T.T @ Q^T keeps softmax denominators computable by an
   in-matmul ones-column (V' has a 65th column of ones -> row 64 of O' = Z)
 - softmax skips max-subtraction (logits are ~N(0,1); exp cannot overflow)
 - causal masking via 0/1 mask multiply on the 4 diagonal-block patterns
 - t-chunk-outer loop interleaves QKV / attention / projection so PE stays
   busy while ACT runs the exps
"""

import os
import sys

for _p in ("/opt/trn_rl_repo", "/root/.axon_site/_ro/trn_rl_repo"):
    if os.path.isdir(_p) and _p not in sys.path:
        sys.path.insert(0, _p)

import numpy as np
from concourse import bacc, mybir, tile
from concourse.bass_utils import run_bass_kernel_spmd

N_CORES = 8
B, T, C = 4, 2048, 1024
H, D = 16, 64          # full model heads
HG = 8                 # heads per core (head-group)
CH = HG * D            # 512, per-core qkv width
NT = T // 128          # 16 s-tiles
NJ = T // 512          # 4 t-chunks
NC_ = C // 128         # 8 contraction tiles
F32 = mybir.dt.float32
F32R = mybir.dt.float32r
AF = mybir.ActivationFunctionType

_CACHE = {}


def _emit(nc, tc, aps):
    xT, wq, wk, wv, wp, bq2, bk2, mask, yout = (
        aps["xT"], aps["wq"], aps["wk"], aps["wv"], aps["wp"],
        aps["bq2"], aps["bk2"], aps["mask"], aps["y"],
    )

    pool = tc.alloc_tile_pool(name="pool", bufs=1)
    psp = tc.alloc_tile_pool(name="ps", bufs=1, space="PSUM")

    # ---- persistent tensors ----
    kt = [pool.tile([128, T], F32R, name=f"kt{m}", tag="kt", bufs=4) for m in range(4)]
    vp = [pool.tile([128, 520], F32R, name=f"vp{i}", tag="vp", bufs=NT)
          for i in range(NT)]
    # single lower-triangle mask (1{s <= t}) for the diagonal 128x128 blocks
    tri = pool.tile([128, 128], F32R, name="tri", tag="tri", bufs=1)
    bqs = pool.tile([128, 4], F32, name="bqs", tag="bias", bufs=2)
    bks = pool.tile([128, 4], F32, name="bks", tag="bias", bufs=2)
    ones = pool.tile([128, 64], F32R, name="ones", tag="ones", bufs=1)
    ones_f = pool.tile([128, 64], F32, name="ones_f", tag="ones_f", bufs=1)

    # weights: wq/wk/wv now, wp reuses the same slots once QKV is done
    W = 24  # shared slot budget for 512-wide weight tiles
    wqs = [pool.tile([128, CH], F32R, name=f"wqs{ci}", tag="w", bufs=W)
           for ci in range(NC_)]
    wks = [pool.tile([128, CH], F32R, name=f"wks{ci}", tag="w", bufs=W)
           for ci in range(NC_)]
    wvs = [pool.tile([128, CH], F32R, name=f"wvs{ci}", tag="w", bufs=W)
           for ci in range(NC_)]
    # DMA queue split (both HWDGE queues; SWDGE descriptor-gen is ~28us per
    # strided tile, so gpsimd is avoided): sync carries wq interleaved with
    # the first x chunk so QT matmuls start immediately; the scalar queue
    # carries wk/wv/bias/mask in parallel.
    xt0 = []
    for ci in range(NC_):
        nc.sync.dma_start(wqs[ci][:], wq[128 * ci:128 * ci + 128, :].bitcast(F32R))
        xt_t = pool.tile([128, 512], F32R, name=f"xt0_{ci}", tag="xt", bufs=8)
        eng = nc.sync if ci < 2 else nc.scalar
        eng.dma_start(
            xt_t[:], xT[128 * ci:128 * ci + 128, 0:512].bitcast(F32R)
        )
        xt0.append(xt_t)
    for ci in range(NC_):
        nc.sync.dma_start(wks[ci][:], wk[128 * ci:128 * ci + 128, :].bitcast(F32R))
    nc.scalar.dma_start(bqs[:], bq2[:])
    nc.scalar.dma_start(bks[:], bk2[:])
    for ci in range(NC_):
        nc.scalar.dma_start(wvs[ci][:], wv[128 * ci:128 * ci + 128, :].bitcast(F32R))
    nc.scalar.dma_start(tri[:], mask[:].bitcast(F32R))
    nc.gpsimd.memset(ones_f[:], 1.0)
    nc.vector.tensor_copy(ones[:], ones_f[:])
    for i in range(NT):
        ocol = vp[i][:, 0:520].rearrange("p (h e) -> p h e", e=65)[:, :, 64:65]
        nc.vector.tensor_copy(ocol, ones_f[:, 0:8].unsqueeze(2))

    qtc = [[None] * NJ for _ in range(4)]   # per-chunk Q^T tiles
    otc = [[None] * NJ for _ in range(4)]   # per-chunk O^T tiles
    wps = [[None, None] for _ in range(4)]  # wp [128,512] halves, loaded late

    def emit_qkv(j):
        if j == 0:
            xts = xt0
        else:
            xts = []
            for ci in range(NC_):
                xt_t = pool.tile([128, 512], F32R, name=f"xt{j}_{ci}", tag="xt",
                                 bufs=8)
                nc.sync.dma_start(
                    xt_t[:],
                    xT[128 * ci:128 * ci + 128, 512 * j:512 * j + 512].bitcast(F32R),
                )
                xts.append(xt_t)
        for wsrc, bias_t, dst, nm in ((wqs, bqs, qtc, "qt"), (wks, bks, None, "kt")):
            for m in range(4):
                ps = psp.tile([128, 512], F32, name=f"{nm}ps{j}_{m}", tag="qk", bufs=2)
                for ci in range(NC_):
                    nc.tensor.matmul(
                        ps[:], wsrc[ci][:, 128 * m:128 * m + 128], xts[ci][:],
                        start=(ci == 0), stop=(ci == NC_ - 1),
                    )
                if dst is None:
                    out_ap = kt[m][:, 512 * j:512 * j + 512]
                else:
                    t_ = pool.tile([128, 512], F32R, name=f"qt{m}_{j}", tag="qtc",
                                   bufs=8)
                    dst[m][j] = t_
                    out_ap = t_[:]
                nc.vector.tensor_scalar_add(out_ap, ps[:], bias_t[:, m:m + 1])
        for u in range(4):
            i = 4 * j + u
            ps = psp.tile([128, 512], F32, name=f"vps{i}", tag="qk", bufs=2)
            for ci in range(NC_):
                nc.tensor.matmul(
                    ps[:], xts[ci][:, 128 * u:128 * u + 128], wvs[ci][:],
                    start=(ci == 0), stop=(ci == NC_ - 1),
                )
            dst = vp[i][:, 0:520].rearrange("p (h e) -> p h e", e=65)[:, :, 0:64]
            src = ps[:].rearrange("p (h e) -> p h e", e=64)
            nc.vector.tensor_copy(dst, src)

    def emit_attn(j, heads=(1, 0, 3, 2, 5, 4, 7, 6)):
        n_i = 4 * j + 4

        def tile_layout(p):
            # pairs of s-tiles per [128,1024] PSUM slot; diagonal tiles are
            # narrowed to the causally valid t-range [128r, 512).
            # entries: (i, slot_col, valid_t0, width, diag_block_col)
            i0, i1 = 2 * p, 2 * p + 1
            r0_, r1_ = i0 - 4 * j, i1 - 4 * j
            if r1_ < 0:
                return [(i0, 0, 0, 512, None), (i1, 512, 0, 512, None)], 1024
            if r0_ == 0:
                return [(i0, 0, 0, 512, 0), (i1, 512, 128, 384, 512)], 896
            return [(i0, 0, 256, 256, 0), (i1, 256, 384, 128, 256)], 384

        # odd heads first: their normalize chain ends in a partition-shifting
        # SBUF->SBUF DMA, so keep an even (cheap-chain) head last
        for h in heads:
            mt = h // 2
            off = 64 * (h % 2)
            ops = psp.tile([65, 512], F32, name=f"ops{h}_{j}", tag="o", bufs=2)
            qsrc = qtc[mt][j][off:off + 64, :]
            for p in range(n_i // 2):
                layout, exp_hi = tile_layout(p)
                sp = psp.tile([128, 1024], F32, name=f"sp{h}_{j}_{p}", tag="sp",
                              bufs=2)
                for (i, scol, t0, w, _) in layout:
                    nc.tensor.matmul(
                        sp[:, scol:scol + w],
                        kt[mt][off:off + 64, 128 * i:128 * i + 128],
                        qsrc[:, t0:t0 + w],
                        start=True, stop=True,
                    )
                et = pool.tile([128, 1024], F32R, name=f"et{h}_{j}_{p}", tag="et",
                               bufs=3)
                nc.scalar.activation(et[:, 0:exp_hi], sp[:, 0:exp_hi], AF.Exp,
                                     scale=0.125)
                for (i, scol, t0, w, dcol) in layout:
                    if dcol is not None:
                        blk = et[:, dcol:dcol + 128]
                        nc.vector.tensor_mul(blk, blk, tri[:])
                    nc.tensor.matmul(
                        ops[:, t0:t0 + w], vp[i][:, 65 * h:65 * h + 65],
                        et[:, scol:scol + w],
                        start=(i == 0), stop=(i == n_i - 1),
                    )
            # normalize: rows 0..63 unnormalized O^T, row 64 = Z
            zr = pool.tile([65, 512], F32R, name=f"zr{h}_{j}", tag="zr", bufs=2)
            nc.vector.tensor_copy(zr[64:65, :], ops[64:65, :])
            rbp = psp.tile([64, 512], F32, name=f"rbp{h}_{j}", tag="o", bufs=2)
            nc.tensor.matmul(rbp[:], ones[64:65, :], zr[64:65, :], start=True,
                             stop=True)
            rbs = pool.tile([64, 512], F32R, name=f"rbs{h}_{j}", tag="rbs", bufs=2)
            with nc.allow_low_precision(reason="fp32r rounding of softmax denom"):
                nc.vector.reciprocal(rbs[:], rbp[:])
            if otc[mt][j] is None:
                otc[mt][j] = pool.tile([128, 512], F32R, name=f"ot{mt}_{j}",
                                       tag="otc", bufs=8)
            if h % 2 == 0:
                nc.vector.tensor_mul(otc[mt][j][0:64, :], ops[0:64, :], rbs[:])
            else:
                st = pool.tile([64, 512], F32R, name=f"st{h}_{j}", tag="st", bufs=1)
                nc.vector.tensor_mul(st[:], ops[0:64, :], rbs[:])
                nc.sync.dma_start(otc[mt][j][64:128, :], st[:])

    def emit_wp_loads():
        for m in range(4):
            for n in range(2):
                t_ = pool.tile([128, 512], F32R, name=f"wps{m}_{n}", tag="w", bufs=W)
                wps[m][n] = t_
                nc.sync.dma_start(
                    t_[:],
                    wp[128 * m:128 * m + 128, 512 * n:512 * n + 512].bitcast(F32R),
                )

    def emit_proj(j):
        for u in range(4):
            t = 4 * j + u
            for n in range(2):
                ps = psp.tile([128, 512], F32, name=f"yps{t}_{n}", tag="qk", bufs=2)
                for m in range(4):
                    nc.tensor.matmul(
                        ps[:], otc[m][j][:, 128 * u:128 * u + 128], wps[m][n][:],
                        start=(m == 0), stop=(m == 3),
                    )
                yo = pool.tile([128, 512], F32, name=f"yo{t}_{n}", tag="yo", bufs=2)
                nc.vector.tensor_copy(yo[:], ps[:])
                nc.sync.dma_start(
                    yout[128 * t:128 * t + 128, 512 * n:512 * n + 512], yo[:]
                )

    emit_qkv(0)
    emit_attn(0)
    emit_qkv(1)
    emit_qkv(2)
    emit_attn(1)
    emit_qkv(3)
    emit_wp_loads()
    emit_attn(2, heads=(1, 0, 3, 2))
    emit_attn(3, heads=(1, 0))
    emit_proj(0)
    emit_attn(2, heads=(5, 4, 7, 6))
    emit_attn(3, heads=(3, 2))
    emit_proj(1)
    emit_attn(3, heads=(5, 4, 7, 6))
    emit_proj(2)
    emit_proj(3)

    for m in range(4):
        qtc[m] = [None] * NJ
        otc[m] = [None] * NJ
    pool.release()
    psp.release()


def build(passes=1):
    key = ("nc", passes)
    if key in _CACHE:
        return _CACHE[key]
    nc = bacc.Bacc("TRN2", target_bir_lowering=False, debug=False,
                   num_devices=N_CORES)
    aps = {
        "xT": nc.dram_tensor("xT", [C, T], F32, kind="ExternalInput").ap(),
        "wq": nc.dram_tensor("wq", [C, CH], F32, kind="ExternalInput").ap(),
        "wk": nc.dram_tensor("wk", [C, CH], F32, kind="ExternalInput").ap(),
        "wv": nc.dram_tensor("wv", [C, CH], F32, kind="ExternalInput").ap(),
        "wp": nc.dram_tensor("wp", [CH, C], F32, kind="ExternalInput").ap(),
        "bq2": nc.dram_tensor("bq2", [128, 4], F32, kind="ExternalInput").ap(),
        "bk2": nc.dram_tensor("bk2", [128, 4], F32, kind="ExternalInput").ap(),
        "mask": nc.dram_tensor("mask", [128, 128], F32, kind="ExternalInput").ap(),
        "y": nc.dram_tensor("y", [T, C], F32, kind="ExternalOutput").ap(),
    }
    with tile.TileContext(nc) as tc:
        for _ in range(passes):
            _emit(nc, tc, aps)
    nc.compile()
    _CACHE[key] = nc
    return nc


def make_in_maps(x, Wq, bq, Wk, bk, Wv, bv, Wp, bp):
    # lower-triangle 0/1 mask for the diagonal 128x128 attention blocks
    s_idx = np.arange(128)[:, None]
    t_idx = np.arange(128)[None, :]
    mask = (s_idx <= t_idx).astype(np.float32)
    in_maps = []
    for c in range(N_CORES):
        b, g = c // 2, c % 2
        cols = slice(CH * g, CH * g + CH)
        in_maps.append({
            "xT": np.ascontiguousarray(x[b].T),
            "wq": np.ascontiguousarray(Wq[:, cols]),
            "wk": np.ascontiguousarray(Wk[:, cols]),
            "wv": np.ascontiguousarray(Wv[:, cols]),
            "wp": np.ascontiguousarray(Wp[cols, :]),
            "bq2": np.ascontiguousarray(bq[cols].reshape(4, 128).T),
            "bk2": np.ascontiguousarray(bk[cols].reshape(4, 128).T),
            "mask": mask,
        })
    return in_maps


def kernel(x, Wq, bq, Wk, bk, Wv, bv, Wp, bp):
    # host-side prep is pure numpy; convert in case jax arrays are passed
    x, Wq, bq, Wk, bk, Wv, bv, Wp, bp = (
        np.asarray(a, dtype=np.float32)
        for a in (x, Wq, bq, Wk, bk, Wv, bv, Wp, bp)
    )
    nc = build()
    in_maps = make_in_maps(x, Wq, bq, Wk, bk, Wv, bv, Wp, bp)
    # the axon-proxied device occasionally reports a transient unrecoverable
    # exec state that clears on a fresh attempt; retry rather than fail
    last_err = None
    for _attempt in range(3):
        try:
            res = run_bass_kernel_spmd(nc, in_maps, core_ids=list(range(N_CORES)))
            break
        except Exception as e:  # noqa: BLE001
            last_err = e
            import time as _time
            _time.sleep(5)
    else:
        raise last_err
    corr = (bv @ Wp + bp).astype(np.float32)
    out = np.empty((B, T, C), dtype=np.float32)
    for b in range(B):
        out[b] = res.results[2 * b]["y"] + res.results[2 * b + 1]["y"] + corr
    return out



# revision 39
# speedup vs baseline: 1.2263x; 1.2263x over previous
"""Causal self-attention (B=4, T=2048, C=1024, H=16, D=64) on 8 trn2 cores.

Sharding: data-parallel over B (4) x tensor-parallel over head-halves (2).
Core c handles batch c//2 with heads [8*(c%2), 8*(c%2)+8). Each core emits a
partial projection output [2048, 1024] in bf16; host sums the two head-half
partials per batch and adds the (bv @ Wp + bp) correction row.

Device layout highlights (all matmul operands bf16; PSUM accumulation fp32):
 - x is pre-transposed + pre-cast to bf16 on host; host also packs
   [Wq | x-chunk0] and [Wk | Wv] so the startup needs few large DMAs
   (the HWDGE dispatch path costs ~630ns per dma_start regardless of size)
 - S^T = K^T.T @ Q^T per 128-row s-tile keeps softmax denominators and the
   P*V product computable entirely in the s-partition layout
 - softmax skips max-subtraction (logits are ~N(0,1); exp cannot overflow)
 - causal masking via 0/1 mask multiply on the diagonal 128x128 blocks
 - O = P.T@V uses moving-V matmuls ([t,65] output per (t-tile, s-tile) pair,
   ones column -> denominator Z lands in column 64), which costs 65 moving
   rows per s-tile instead of the 128-512 of the t-moving formulation
 - normalization is a per-partition reciprocal+scale (t is the partition dim
   after the O flip), then a PE transpose pairs two heads back into the
   [d, t] layout the projection needs
 - the attention phases are exp(ACT)-bound, so the whole kernel is emitted
   as one interleaved stream: attention of chunk j starts as soon as its own
   Q/K chains are out (fine-grained gating on the QKV generator), and QKV /
   projection matmul chains are pumped between attention pairs as PE filler
   under an ACT-clock model (pump until emitted-PE-ns catches the emitted
   exp-ns), which keeps both engines dense and saves filler for the endgame
"""

import os
import sys

for _p in ("/opt/trn_rl_repo", "/root/.axon_site/_ro/trn_rl_repo"):
    if os.path.isdir(_p) and _p not in sys.path:
        sys.path.insert(0, _p)

import numpy as np
import ml_dtypes
from concourse import bacc, mybir, tile
from concourse.bass_utils import run_bass_kernel_spmd

N_CORES = 8
B, T, C = 4, 2048, 1024
H, D = 16, 64          # full model heads
HG = 8                 # heads per core (head-group)
CH = HG * D            # 512, per-core qkv width
NT = T // 128          # 16 s-tiles
NJ = T // 512          # 4 t-chunks
NC_ = C // 128         # 8 contraction tiles
F32 = mybir.dt.float32
BF16 = mybir.dt.bfloat16
AF = mybir.ActivationFunctionType
BF16NP = ml_dtypes.bfloat16

PE_NS = 1.0 / 2.4      # ns per matmul moving row at full p-state
ACT_NS = 1.0 / 1.2     # ns per activation element (per partition)
ACT_FIX = 235          # per-activation-instruction overhead (access + sems)
PAIR_MARGIN = 250      # extra PE-ns of filler per pair (mask + sem latency)

_CACHE = {}


def _emit(nc, tc, aps):
    wqx0, wkv, xr, wp, bqk, msk, yout = (
        aps["wqx0"], aps["wkv"], aps["xr"], aps["wp"], aps["bqk"],
        aps["msk"], aps["y"],
    )

    pool = tc.alloc_tile_pool(name="pool", bufs=1)
    psp = tc.alloc_tile_pool(name="ps", bufs=1, space="PSUM")

    # ---- persistent tensors ----
    kt = [pool.tile([128, T], BF16, name=f"kt{m}", tag="kt", bufs=4) for m in range(4)]
    vp = [pool.tile([128, 520], BF16, name=f"vp{i}", tag="vp", bufs=NT)
          for i in range(NT)]
    bqks = pool.tile([128, 8], F32, name="bqks", tag="bias", bufs=1)
    msks = pool.tile([128, 256], BF16, name="msks", tag="msk", bufs=1)
    tri = msks[:, 0:128]    # lower-triangle 1{s <= t} mask for diag blocks
    idn = msks[:, 128:256]  # identity for the PE transposes
    ones_f = pool.tile([128, 8], BF16, name="ones_f", tag="ones_f", bufs=1)

    # packed input slabs: [Wq | x-chunk0], [Wk | Wv], x chunks 1-3, Wp
    wqxs = [pool.tile([128, 1024], BF16, name=f"wqx{ci}", tag="wqx", bufs=NC_)
            for ci in range(NC_)]
    wkvs = [pool.tile([128, 1024], BF16, name=f"wkv{ci}", tag="wkv", bufs=NC_)
            for ci in range(NC_)]
    xrs = [pool.tile([128, 1536], BF16, name=f"xr{ci}", tag="xr", bufs=NC_)
           for ci in range(NC_)]
    wpt = [pool.tile([128, 1024], BF16, name=f"wpt{m}", tag="wp", bufs=4)
           for m in range(4)]
    wqs = [t[:, 0:512] for t in wqxs]
    wks = [t[:, 0:512] for t in wkvs]
    wvs = [t[:, 512:1024] for t in wkvs]

    def xts_of(j):
        if j == 0:
            return [t[:, 512:1024] for t in wqxs]
        return [t[:, 512 * (j - 1):512 * j] for t in xrs]

    # DMA queue split (SP and ACT own the two HWDGE queues): sync carries
    # [Wq|x0] then the x remainder and later the y stores; the scalar queue
    # carries biases/masks, [Wk|Wv], and Wp
    for ci in range(NC_):
        nc.sync.dma_start(wqxs[ci][:], wqx0[128 * ci:128 * ci + 128, :])
    nc.scalar.dma_start(bqks[:], bqk[:])
    nc.scalar.dma_start(msks[:], msk[:])
    for ci in range(NC_):
        nc.scalar.dma_start(wkvs[ci][:], wkv[128 * ci:128 * ci + 128, :])
    for m in range(4):
        nc.scalar.dma_start(wpt[m][:], wp[128 * m:128 * m + 128, :])
    for ci in range(NC_):
        nc.sync.dma_start(xrs[ci][:], xr[128 * ci:128 * ci + 128, :])
    nc.gpsimd.memset(ones_f[:], 1.0)
    for i in range(NT):
        ocol = vp[i][:, 0:520].rearrange("p (h e) -> p h e", e=65)[:, :, 64:65]
        nc.vector.tensor_copy(ocol, ones_f[:, 0:8].unsqueeze(2))

    qtc = [[None] * NJ for _ in range(4)]   # per-chunk Q^T tiles
    otc = [[None] * NJ for _ in range(4)]   # per-chunk O^T tiles (proj input)
    otn = {}                                # (m, j, u) -> normalized-O tile

    # ---- PE filler scheduling ------------------------------------------
    # generators yield PE row counts after each emitted matmul (ints) and
    # readiness markers (tuples); the attention pair loop pumps them to fill
    # exp-latency bubbles, and gates on the markers. agens carry the deferred
    # P*V accumulation chains of the previous head (highest priority: their
    # PSUM slots and et tiles gate upcoming heads), qgens (QKV work) come
    # next and carry the gating markers, bgens (projection work) last.
    agens = []
    qgens = []
    bgens = []
    reached = set()
    clock = {"pe": 0.0, "act": 0.0, "qrows": 0}

    def pe(rows):
        clock["pe"] += rows * PE_NS

    def add_qgen(gen, rows):
        qgens.append(gen)
        clock["qrows"] += rows

    def _advance_a():
        while agens:
            try:
                y = next(agens[0])
            except StopIteration:
                agens.pop(0)
                continue
            pe(y)
            return y
        return None

    def _advance_q():
        y = _advance_a()
        if y is not None:
            return y
        while qgens:
            try:
                y = next(qgens[0])
            except StopIteration:
                qgens.pop(0)
                continue
            if isinstance(y, tuple):
                reached.add(y)
                continue
            pe(y)
            clock["qrows"] -= y
            return y
        return None

    def _advance():
        y = _advance_q()
        if y is not None:
            return y
        while bgens:
            try:
                y = next(bgens[0])
            except StopIteration:
                bgens.pop(0)
                continue
            if isinstance(y, tuple):
                reached.add(y)
                continue
            pe(y)
            return y
        return None

    def pump_to(target_pe_ns):
        while clock["pe"] < target_pe_ns:
            if _advance() is None:
                return

    def pump_qrows(rows):
        while rows > 0:
            y = _advance_q()
            if y is None:
                return
            rows -= y

    def pump_until(mark):
        while agens:
            _advance_a()
        while mark not in reached and qgens:
            _advance_q()

    def drain():
        while _advance() is not None:
            pass

    def gen_qkv(j):
        xts = xts_of(j)

        def v_chain(u):
            i = 4 * j + u
            ps = psp.tile([128, 512], F32, name=f"vps{i}", tag="qk", bufs=2)
            for ci in range(NC_):
                nc.tensor.matmul(
                    ps[:], xts[ci][:, 128 * u:128 * u + 128], wvs[ci][:],
                    start=(ci == 0), stop=(ci == NC_ - 1),
                )
                yield 512
            dst = vp[i][:, 0:520].rearrange("p (h e) -> p h e", e=65)[:, :, 0:64]
            src = ps[:].rearrange("p (h e) -> p h e", e=64)
            nc.vector.tensor_copy(dst, src)

        def qk_chain(m, wsrc, bias_col, dst):
            ps = psp.tile([128, 512], F32, name=f"qkps{j}_{m}", tag="qk", bufs=2)
            for ci in range(NC_):
                nc.tensor.matmul(
                    ps[:], wsrc[ci][:, 128 * m:128 * m + 128], xts[ci][:],
                    start=(ci == 0), stop=(ci == NC_ - 1),
                )
                yield 512
            if dst is None:
                out_ap = kt[m][:, 512 * j:512 * j + 512]
            else:
                t_ = pool.tile([128, 512], BF16, name=f"qt{m}_{j}", tag="qtc",
                               bufs=10)
                dst[m][j] = t_
                out_ap = t_[:]
            nc.vector.tensor_scalar_add(out_ap, ps[:], bqks[:, bias_col:bias_col + 1])

        if j == 0:
            # startup: the [Wq|x0] / [Wk|Wv] slabs trickle in one DMA at a
            # time, so emit Q0/Q1/K0/K1 ci-outer (four open chains; the K
            # chains borrow the idle sp-tag PSUM banks) so every arriving
            # slab feeds four matmuls instead of one
            def wave(ma, mb):
                pq = [psp.tile([128, 512], F32, name=f"q0ps{m}", tag="qk",
                               bufs=2) for m in (ma, mb)]
                pk = [psp.tile([128, 1024], F32, name=f"k0ps{m}", tag="sp",
                               bufs=2) for m in (ma, mb)]
                for ci in range(NC_):
                    for k, m in enumerate((ma, mb)):
                        nc.tensor.matmul(
                            pq[k][:], wqs[ci][:, 128 * m:128 * m + 128],
                            xts[ci][:], start=(ci == 0), stop=(ci == NC_ - 1),
                        )
                        yield 512
                        nc.tensor.matmul(
                            pk[k][:, 0:512], wks[ci][:, 128 * m:128 * m + 128],
                            xts[ci][:], start=(ci == 0), stop=(ci == NC_ - 1),
                        )
                        yield 512
                for k, m in enumerate((ma, mb)):
                    t_ = pool.tile([128, 512], BF16, name=f"qt{m}_0", tag="qtc",
                                   bufs=10)
                    qtc[m][0] = t_
                    nc.vector.tensor_scalar_add(t_[:], pq[k][:],
                                                bqks[:, 4 + m:5 + m])
                    nc.vector.tensor_scalar_add(kt[m][:, 0:512], pk[k][:, 0:512],
                                                bqks[:, m:m + 1])

            yield from wave(0, 1)
            for u in range(4):
                yield from v_chain(u)
            yield ("q", 0, 0)
            yield ("q", 0, 1)
            yield from wave(2, 3)
            yield ("q", 0, 2)
            yield ("q", 0, 3)
        else:
            # steady state: V first (attention on this chunk needs all of V),
            # then per-m K and Q so attention head-pairs can gate finely
            for u in range(4):
                yield from v_chain(u)
            for m in range(4):
                yield from qk_chain(m, wks, m, None)
                yield from qk_chain(m, wqs, 4 + m, qtc)
                yield ("q", j, m)

    def gen_proj(j):
        for u in range(4):
            t = 4 * j + u
            yo = pool.tile([128, 1024], BF16, name=f"yo{t}", tag="yo", bufs=3)
            for n in range(2):
                ps = psp.tile([128, 512], F32, name=f"yps{t}_{n}", tag="qk",
                              bufs=2)
                for m in range(4):
                    nc.tensor.matmul(
                        ps[:], otc[m][j][:, 128 * u:128 * u + 128],
                        wpt[m][:, 512 * n:512 * n + 512],
                        start=(m == 0), stop=(m == 3),
                    )
                    yield 512
                nc.vector.tensor_copy(yo[:, 512 * n:512 * n + 512], ps[:])
            nc.sync.dma_start(yout[128 * t:128 * t + 128, :], yo[:])

    # chunk 3's projection is split per-m with an SBUF accumulator: each m's
    # matmuls become endgame filler as soon as that head-pair finishes, and
    # only the m=3 batch remains in the post-attention tail. That final pass
    # is split across engines: units u=2,3 merge accumulator+psum on DVE,
    # while u=0,1 ship their m0-2 partial to DRAM in f32 (host adds it) and
    # ACT -- idle once the exps are done -- copies the m3 psum out.
    ysb = [pool.tile([128, 1024], F32, name=f"ysb{u}", tag="ysb", bufs=4)
           for u in range(4)]
    yo3 = [pool.tile([128, 1024], BF16, name=f"yo3_{u}", tag="yo3", bufs=4)
           for u in range(4)]
    y3a = aps["y3a"]

    def gen_proj3_m(m):
        for u in range(4):
            for n in range(2):
                # the m=3 batch runs post-attention: the sp-tag PSUM banks
                # are free by then, so the ACT-path units borrow them and
                # all eight units' matmuls can be in flight at once
                if m == 3 and u < 2:
                    ps = psp.tile([128, 1024], F32, name=f"y3ps{m}_{u}_{n}",
                                  tag="sp", bufs=2)[:, 0:512]
                else:
                    ps = psp.tile([128, 512], F32, name=f"y3ps{m}_{u}_{n}",
                                  tag="qk", bufs=2)[:]
                nc.tensor.matmul(
                    ps, otc[m][3][:, 128 * u:128 * u + 128],
                    wpt[m][:, 512 * n:512 * n + 512], start=True, stop=True,
                )
                yield 512
                acc = ysb[u][:, 512 * n:512 * n + 512]
                if m == 0:
                    nc.vector.tensor_copy(acc, ps)
                elif m < 3:
                    nc.vector.tensor_add(acc, acc, ps)
                elif u >= 2:
                    nc.vector.tensor_add(
                        yo3[u][:, 512 * n:512 * n + 512], acc, ps
                    )
                else:
                    nc.scalar.activation(
                        yo3[u][:, 512 * n:512 * n + 512], ps, AF.Copy
                    )
            if m == 2 and u < 2:
                nc.sync.dma_start(y3a[128 * u:128 * u + 128, :], ysb[u][:])
            if m == 3:
                nc.sync.dma_start(yout[128 * (12 + u):128 * (13 + u), :],
                                  yo3[u][:])

    def tile_layout(p, j):
        # pairs of s-tiles per [128,1024] PSUM slot; diagonal tiles are
        # narrowed to the causally valid t-range [128r, 512).
        # entries: (i, slot_col, valid_t0, width, diag_block_col)
        i0, i1 = 2 * p, 2 * p + 1
        r0_, r1_ = i0 - 4 * j, i1 - 4 * j
        if r1_ < 0:
            return [(i0, 0, 0, 512, None), (i1, 512, 0, 512, None)], 1024
        if r0_ == 0:
            return [(i0, 0, 0, 512, 0), (i1, 512, 128, 384, 512)], 896
        return [(i0, 0, 256, 256, 0), (i1, 256, 384, 128, 256)], 384

    def emit_s_pair(j, h, p):
        """S matmuls + exp + mask for pair p; returns (et, layout) for the
        deferred P*V chains."""
        mt, off = h // 2, 64 * (h % 2)
        layout, exp_hi = tile_layout(p, j)
        qsrc = qtc[mt][j][off:off + 64, :]
        sp = psp.tile([128, 1024], F32, name=f"sp{h}_{j}_{p}", tag="sp", bufs=2)
        for (i, scol, t0, w, _) in layout:
            nc.tensor.matmul(
                sp[:, scol:scol + w],
                kt[mt][off:off + 64, 128 * i:128 * i + 128],
                qsrc[:, t0:t0 + w],
                start=True, stop=True,
            )
            pe(w)
        et = pool.tile([128, 1024], BF16, name=f"et{h}_{j}_{p}", tag="et", bufs=24)
        nc.scalar.activation(et[:, 0:exp_hi], sp[:, 0:exp_hi], AF.Exp, scale=0.125)
        clock["act"] = max(clock["act"], clock["pe"]) + exp_hi * ACT_NS + ACT_FIX
        for (i, scol, t0, w, dcol) in layout:
            if dcol is not None:
                blk = et[:, dcol:dcol + 128]
                nc.vector.tensor_mul(blk, blk, tri)
        return et, layout

    def make_chain_gen(j, h, ops, pairs, tph):
        """Deferred P*V for head h: the four 65-wide accumulation chains of
        `ops` emitted one chain at a time (a chain's matmuls must never
        interleave with other matmuls in the same PSUM bank — the
        accumulation context is per-bank), each followed by its normalize
        O[t, d] / Z[t] (t is the partition dim so Z is a per-partition
        scalar). Odd heads transpose the paired heads back to [d, t] and
        assemble otc[m][j]. Pumped into the next head's pair stream."""
        m, off = h // 2, 64 * (h % 2)
        for u in range(4):
            last_i = 4 * j + u
            for et, layout in pairs:
                for (i, scol, t0, w, _) in layout:
                    if 128 * u < t0 or i > last_i:
                        continue
                    col = scol + 128 * u - t0
                    nc.tensor.matmul(
                        ops[:, 65 * u:65 * u + 65],
                        et[:, col:col + 128],
                        vp[i][:, 65 * h:65 * h + 65],
                        start=(i == 0), stop=(i == last_i),
                    )
                    yield 65
            key = (m, j, u)
            if key not in otn:
                otn[key] = pool.tile([128, 128], BF16, name=f"on{m}_{j}_{u}",
                                     tag="otn", bufs=10)
            rz = pool.tile([128, 1], F32, name=f"rz{h}_{j}_{u}", tag="rz",
                           bufs=8)
            nc.vector.reciprocal(rz[:], ops[:, 65 * u + 64:65 * u + 65])
            nc.vector.tensor_scalar_mul(
                otn[key][:, off:off + 64], ops[:, 65 * u:65 * u + 64], rz[:]
            )
            if h % 2 == 1:
                if tph[0] is None:
                    # same pool tag as the ops tiles; allocating it only
                    # after ops readers have been emitted keeps the in-order
                    # engine streams deadlock-free
                    tph[0] = psp.tile([128, 512], BF16, name=f"tp{m}_{j}",
                                      tag="o", bufs=2)
                nc.tensor.transpose(
                    tph[0][:, 128 * u:128 * u + 128], otn[key][:], idn
                )
                yield 128
                otn[key] = None
        if h % 2 == 1:
            t_ = pool.tile([128, 512], BF16, name=f"ot{m}_{j}", tag="otc",
                           bufs=8)
            otc[m][j] = t_
            nc.vector.tensor_copy(t_[:], tph[0][:])

    # ---- the interleaved attention stream ------------------------------
    # all 32 (chunk, head) units form ONE software-pipelined pair stream:
    # each head's S/exp/mask pairs stream with pumped filler covering the
    # exp latency, its deferred P*V chain generator is pumped into the NEXT
    # head's pairs, and the pipeline flows across chunk boundaries so the
    # ACT stream never drains at a chunk gate
    QKV_ROWS = 12 * NC_ * 512
    add_qgen(gen_qkv(0), QKV_ROWS)
    pump_until(("q", 0, 0))
    add_qgen(gen_qkv(1), QKV_ROWS)
    first = True
    for j in range(NJ):
        n_p = (4 * j + 4) // 2
        pairs_left = 8 * n_p
        for m in range(4):
            if j > 0 or m > 0:
                pump_until(("q", j, m))
            for h in (2 * m, 2 * m + 1):
                # leave at most the previous head's chain generator pending
                # before claiming an O-accumulator slot (bufs=2)
                while len(agens) > 1:
                    _advance_a()
                # O accumulator: 4 chains of [t=128, 65] at cols 65u..65u+65;
                # column 64 of each chain = softmax denominator Z
                ops = psp.tile([128, 512], F32, name=f"ops{h}_{j}", tag="o",
                               bufs=2)
                tph = [None] if h % 2 == 0 else tph  # noqa: F821
                pairs = []
                for p in range(n_p):
                    gate = clock["act"]
                    pairs.append(emit_s_pair(j, h, p))
                    if not first:
                        # spread the gated QKV work of upcoming chunks evenly
                        # across this chunk's pairs, then top up with any
                        # filler (the previous head's P*V first) until the
                        # PE stream catches the ACT clock
                        pump_qrows(-(-clock["qrows"] // max(pairs_left, 1)))
                        pump_to(gate + PAIR_MARGIN)
                    first = False
                    pairs_left -= 1
                agens.append(make_chain_gen(j, h, ops, pairs, tph))
                if j == 3 and h % 2 == 1:
                    # chunk 3's projection flows in per head-pair, right
                    # behind that pair's chain generator
                    bgens.append(gen_proj3_m(m))
        # after chunk j, its projection becomes background filler and the
        # chunk-after-next's QKV joins the queue
        if j < 3:
            bgens.append(gen_proj(j))
            if j + 2 < NJ:
                add_qgen(gen_qkv(j + 2), QKV_ROWS)
    drain()

    for m in range(4):
        qtc[m] = [None] * NJ
        otc[m] = [None] * NJ
    pool.release()
    psp.release()


def build(passes=1):
    key = ("nc", passes)
    if key in _CACHE:
        return _CACHE[key]
    nc = bacc.Bacc("TRN2", target_bir_lowering=False, debug=False,
                   num_devices=N_CORES)
    aps = {
        "wqx0": nc.dram_tensor("wqx0", [C, 1024], BF16, kind="ExternalInput").ap(),
        "wkv": nc.dram_tensor("wkv", [C, 1024], BF16, kind="ExternalInput").ap(),
        "xr": nc.dram_tensor("xr", [C, 1536], BF16, kind="ExternalInput").ap(),
        "wp": nc.dram_tensor("wp", [CH, C], BF16, kind="ExternalInput").ap(),
        "bqk": nc.dram_tensor("bqk", [128, 8], F32, kind="ExternalInput").ap(),
        "msk": nc.dram_tensor("msk", [128, 256], BF16, kind="ExternalInput").ap(),
        "y": nc.dram_tensor("y", [T, C], BF16, kind="ExternalOutput").ap(),
        "y3a": nc.dram_tensor("y3a", [256, C], F32, kind="ExternalOutput").ap(),
    }
    with tile.TileContext(nc) as tc:
        for _ in range(passes):
            _emit(nc, tc, aps)
    nc.compile()
    _CACHE[key] = nc
    return nc


def make_in_maps(x, Wq, bq, Wk, bk, Wv, bv, Wp, bp):
    # lower-triangle 0/1 mask for the diagonal 128x128 attention blocks,
    # packed beside the transpose identity
    s_idx = np.arange(128)[:, None]
    t_idx = np.arange(128)[None, :]
    msk = np.concatenate(
        [(s_idx <= t_idx).astype(BF16NP), np.eye(128, dtype=BF16NP)], axis=1
    )
    in_maps = []
    for c in range(N_CORES):
        b, g = c // 2, c % 2
        cols = slice(CH * g, CH * g + CH)
        xT = np.ascontiguousarray(x[b].T).astype(BF16NP)
        # bias columns: [bk m-blocks | bq m-blocks], each [128, 4]
        bqk = np.concatenate(
            [bk[cols].reshape(4, 128).T, bq[cols].reshape(4, 128).T], axis=1
        ).astype(np.float32)
        in_maps.append({
            "wqx0": np.concatenate(
                [Wq[:, cols].astype(BF16NP), xT[:, 0:512]], axis=1
            ),
            "wkv": np.concatenate(
                [Wk[:, cols].astype(BF16NP), Wv[:, cols].astype(BF16NP)], axis=1
            ),
            "xr": np.ascontiguousarray(xT[:, 512:2048]),
            "wp": np.ascontiguousarray(Wp[cols, :]).astype(BF16NP),
            "bqk": np.ascontiguousarray(bqk),
            "msk": np.ascontiguousarray(msk),
        })
    return in_maps


def kernel(x, Wq, bq, Wk, bk, Wv, bv, Wp, bp):
    # host-side prep is pure numpy; convert in case jax arrays are passed
    x, Wq, bq, Wk, bk, Wv, bv, Wp, bp = (
        np.asarray(a, dtype=np.float32)
        for a in (x, Wq, bq, Wk, bk, Wv, bv, Wp, bp)
    )
    nc = build()
    in_maps = make_in_maps(x, Wq, bq, Wk, bk, Wv, bv, Wp, bp)
    # the axon-proxied device occasionally reports a transient unrecoverable
    # exec state that clears on a fresh attempt; retry rather than fail
    last_err = None
    for _attempt in range(3):
        try:
            res = run_bass_kernel_spmd(nc, in_maps, core_ids=list(range(N_CORES)))
            break
        except Exception as e:  # noqa: BLE001
            last_err = e
            import time as _time
            _time.sleep(5)
    else:
        raise last_err
    corr = (bv @ Wp + bp).astype(np.float32)
    out = np.empty((B, T, C), dtype=np.float32)
    for b in range(B):
        ra, rb = res.results[2 * b], res.results[2 * b + 1]
        out[b] = (np.asarray(ra["y"], dtype=np.float32)
                  + np.asarray(rb["y"], dtype=np.float32)
                  + corr)
        # rows 1536:1792 carry only the m=3 projection partial in y; the
        # m0-2 partial for them ships separately in f32
        out[b, 1536:1792] += (np.asarray(ra["y3a"], dtype=np.float32)
                              + np.asarray(rb["y3a"], dtype=np.float32))
    return out
